# revision 15
# baseline (speedup 1.0000x reference)
import sys

sys.path.insert(0, "/opt/trn_rl_repo")

from contextlib import ExitStack

import numpy as np

import concourse.bacc as bacc
import concourse.mybir as mybir
import concourse.tile as tile
from concourse.bass_utils import run_bass_kernel_spmd

N_CORES = 8
B, S, D = 2, 2048, 1024
NG, HD = 4, 64            # kv groups, head dim
HPG = 4                   # q heads per group
GW = HPG * HD             # 256 q-proj width per group
QKVW = GW + 2 * HD        # 384 per-group fused qkv width
ROPE_THETA = 50000.0
SCALE = 1.0 / np.sqrt(HD)
F32R = mybir.dt.float32r
F32 = mybir.dt.float32

_BUILD_CACHE = {}


def _rope_tables():
    inv_freq = 1.0 / (ROPE_THETA ** (np.arange(0, HD, 2, dtype=np.float64) / HD))
    freqs = np.arange(S, dtype=np.float64)[:, None] * inv_freq[None, :]   # [S, 32]
    emb = np.concatenate([freqs, freqs], axis=-1)                          # [S, 64]
    cosT = np.cos(emb).T.astype(np.float32)                                # [64, S]
    sinT = np.sin(emb).T.astype(np.float32)
    return cosT, sinT


def _rot_matrix():
    # rot_half(x)[d] = -x[d+32] (d<32) ; x[d-32] (d>=32)
    R = np.zeros((HD, HD), dtype=np.float32)
    R[np.arange(32), np.arange(32) + 32] = -1.0
    R[np.arange(32, 64), np.arange(32, 64) - 32] = 1.0
    # device computes psum_rot = lhsT.T @ q  with lhsT[k, d] = R[d, k]
    RT = R.T.copy()
    out = np.zeros((128, 128), dtype=np.float32)
    out[:64, :64] = RT
    out[64:, 64:] = RT
    return out


def _build():
    if "nc" in _BUILD_CACHE:
        return _BUILD_CACHE["nc"]
    nc = bacc.Bacc(None, target_bir_lowering=False, debug=False, num_devices=N_CORES)

    xT = nc.dram_tensor("xT", [D, S], F32R, kind="ExternalInput").ap()
    wqkv = nc.dram_tensor("wqkv", [D, QKVW], F32R, kind="ExternalInput").ap()
    wout = nc.dram_tensor("wout", [GW, D], F32R, kind="ExternalInput").ap()
    cos2 = nc.dram_tensor("cos2", [128, S], F32R, kind="ExternalInput").ap()
    sin2 = nc.dram_tensor("sin2", [128, S], F32R, kind="ExternalInput").ap()
    rt2 = nc.dram_tensor("rt2", [128, 128], F32R, kind="ExternalInput").ap()
    ident2 = nc.dram_tensor("ident2", [128, HD], F32R, kind="ExternalInput").ap()
    onesc = nc.dram_tensor("onesc", [1, HD], F32R, kind="ExternalInput").ap()
    onesr = nc.dram_tensor("onesr", [128, 16], F32R, kind="ExternalInput").ap()
    trimask = nc.dram_tensor("trimask", [128, 128], F32R, kind="ExternalInput").ap()
    out_d = nc.dram_tensor("out", [S, D], F32, kind="ExternalOutput").ap()
    dbg_qhat0 = nc.dram_tensor("dbg_qhat0", [128, S], F32, kind="ExternalOutput").ap()
    dbg_kt2 = nc.dram_tensor("dbg_kt2", [64, S], F32, kind="ExternalOutput").ap()
    dbg_vall = nc.dram_tensor("dbg_vall", [128, 16 * 65], F32, kind="ExternalOutput").ap()
    dbg_ot0 = nc.dram_tensor("dbg_ot0", [64, S], F32, kind="ExternalOutput").ap()
    dbg_pt0 = nc.dram_tensor("dbg_pt0", [128, 1024], F32, kind="ExternalOutput").ap()
    dbg_osb0 = nc.dram_tensor("dbg_osb0", [65, 1024], F32, kind="ExternalOutput").ap()

    NT = S // 128    # 16 sk tiles
    with tile.TileContext(nc) as tc, ExitStack() as ctx, \
            nc.allow_low_precision(reason="f32r (tf32-like) compute throughout"):
        cpool = ctx.enter_context(tc.tile_pool(name="consts", bufs=1))
        persist = ctx.enter_context(tc.tile_pool(name="persist", bufs=1))
        actx = ExitStack()
        xpool = actx.enter_context(tc.tile_pool(name="x", bufs=1))
        apool = actx.enter_context(tc.tile_pool(name="aconsts", bufs=1))
        rope = actx.enter_context(tc.tile_pool(name="rope", bufs=2))

        ps_qkv = actx.enter_context(tc.tile_pool(name="ps_qkv", bufs=2, space="PSUM"))
        ps_rot = actx.enter_context(tc.tile_pool(name="ps_rot", bufs=2, space="PSUM"))

        # ---- load constants / inputs ----
        w_t = []
        for k in range(8):
            t = apool.tile([128, QKVW], F32R, name=f"wq{k}", tag=f"wq{k}")
            nc.sync.dma_start(t[:], wqkv[k * 128:(k + 1) * 128, :])
            w_t.append(t)
        wo_t = []
        for h in range(HPG):
            t = cpool.tile([HD, D], F32R, name=f"wo{h}", tag=f"wo{h}")
            nc.sync.dma_start(t[:], wout[h * HD:(h + 1) * HD, :])
            wo_t.append(t)
        cos_t = apool.tile([128, S], F32R, tag="cos")
        nc.sync.dma_start(cos_t[:], cos2)
        sin_t = apool.tile([128, S], F32R, tag="sin")
        nc.sync.dma_start(sin_t[:], sin2)
        rt_t = cpool.tile([128, 128], F32R, tag="rt2")
        nc.sync.dma_start(rt_t[:], rt2)
        id_t = cpool.tile([128, HD], F32R, tag="id")
        nc.sync.dma_start(id_t[:], ident2)
        oc_t = cpool.tile([1, HD], F32R, tag="onesc")
        nc.sync.dma_start(oc_t[:], onesc)
        or_t = cpool.tile([128, 16], F32R, tag="onesr")
        nc.sync.dma_start(or_t[:], onesr)
        tm_t = cpool.tile([128, 128], F32R, tag="trim")
        nc.sync.dma_start(tm_t[:], trimask)

        x_t = []
        for k in range(8):
            t = xpool.tile([128, S], F32R, name=f"xt{k}", tag=f"xt{k}")
            nc.sync.dma_start(t[:], xT[k * 128:(k + 1) * 128, :])
            x_t.append(t)

        # persistent activations
        qhat = [persist.tile([128, S], F32R, name=f"qhat{i}", tag=f"qhat{i}") for i in range(2)]
        kt2 = persist.tile([64, S], F32R, tag="kt2")
        kv_sb = persist.tile([128, S], F32R, tag="kvsb")
        v_all = persist.tile([128, NT * (HD + 1)], F32R, tag="vall")

        # ---- stage A: fused QKV projection + RoPE ----
        for m in range(3):
            for n in range(4):
                cl, cw = n * 512, 512
                pq = ps_qkv.tile([128, 512], F32)
                for k in range(8):
                    nc.tensor.matmul(
                        pq[:], w_t[k][:, m * 128:(m + 1) * 128],
                        x_t[k][:, cl:cl + cw],
                        start=(k == 0), stop=(k == 7),
                    )
                if m < 2:
                    qraw = rope.tile([128, 512], F32R, tag="qraw")
                    nc.vector.tensor_scalar_add(qraw[:], pq[:], 0.0)
                    pr = ps_rot.tile([128, 512], F32)
                    nc.tensor.matmul(pr[:], rt_t[:], qraw[:], start=True, stop=True)
                    qrot = rope.tile([128, 512], F32R, tag="qrot")
                    nc.vector.tensor_scalar_add(qrot[:], pr[:], 0.0)
                    dst = qhat[m][:, cl:cl + cw]
                    nc.vector.tensor_mul(dst, qraw[:], cos_t[:, cl:cl + cw])
                    tsin = rope.tile([128, 512], F32R, tag="tsin")
                    nc.vector.tensor_mul(tsin[:], qrot[:], sin_t[:, cl:cl + cw])
                    nc.vector.tensor_add(dst, dst, tsin[:])
                else:
                    nc.vector.tensor_scalar_add(kv_sb[:, cl:cl + cw], pq[:], 0.0)
                    pr = ps_rot.tile([128, 512], F32)
                    nc.tensor.matmul(
                        pr[:64], rt_t[:64, :64], kv_sb[:64, cl:cl + cw],
                        start=True, stop=True,
                    )
                    krot = rope.tile([64, 512], F32R, tag="qrot")
                    nc.vector.tensor_scalar_add(krot[:], pr[:64], 0.0)
                    kdst = kt2[:64, cl:cl + cw]
                    nc.vector.tensor_mul(kdst, kv_sb[:64, cl:cl + cw], cos_t[:64, cl:cl + cw])
                    tsin = rope.tile([64, 512], F32R, tag="tsin")
                    nc.vector.tensor_mul(tsin[:], krot[:], sin_t[:64, cl:cl + cw])
                    nc.vector.tensor_add(kdst, kdst, tsin[:])

        # stage odd heads' Q and V rows to base-partition-0 tiles (engines
        # cannot shift partitions; matmul operands must share base partition)
        actx.close()
        persc = ctx.enter_context(tc.tile_pool(name="persc", bufs=1))
        qodd = [persc.tile([HD, S], F32R, name=f"qodd{i}", tag=f"qodd{i}") for i in range(2)]
        v_nat = persc.tile([HD, S], F32R, tag="vnat")
        ot_h = [persc.tile([HD, S], F32R, name=f"ot{h}", tag=f"ot{h}") for h in range(HPG)]
        nc.sync.dma_start(qodd[0][:], qhat[0][64:, :])
        nc.sync.dma_start(qodd[1][:], qhat[1][64:, :])
        nc.sync.dma_start(v_nat[:], kv_sb[64:, :])
        ptp = ctx.enter_context(tc.tile_pool(name="pt", bufs=3))
        osb_p = ctx.enter_context(tc.tile_pool(name="osb", bufs=2))
        rb_p = ctx.enter_context(tc.tile_pool(name="rb", bufs=2))
        oev_p = ctx.enter_context(tc.tile_pool(name="oev", bufs=2))
        ps_s = ctx.enter_context(tc.tile_pool(name="ps_s", bufs=2, space="PSUM"))
        ps_o = ctx.enter_context(tc.tile_pool(name="ps_o", bufs=1, space="PSUM"))
        ps_sm = ctx.enter_context(tc.tile_pool(name="ps_sm", bufs=2, space="PSUM"))

        # ---- stage B: V natural + ones column ----
        for st in range(NT):
            pv = ps_sm.tile([128, HD], F32R, tag="sm")
            nc.tensor.transpose(pv[:], v_nat[:, st * 128:(st + 1) * 128], id_t[:64, :])
            nc.vector.tensor_scalar_add(
                v_all[:, st * (HD + 1):st * (HD + 1) + HD], pv[:], 0.0
            )
        ones_dst = v_all[:].rearrange("p (t c) -> p t c", c=HD + 1)[:, :, HD:HD + 1]
        ones_src = or_t[:].rearrange("p (t c) -> p t c", c=1)
        nc.vector.tensor_scalar_add(ones_dst, ones_src, 0.0)

        # ---- stage C: attention (per head, per 1024-wide sq chunk) ----
        for h in range(HPG):
            qh = qhat[h // 2][:64, :] if h % 2 == 0 else qodd[h // 2][:]
            for c2 in range(2):
                sq0 = c2 * 1024
                po = ps_o.tile([65, 1024], F32)
                n_t = 8 * c2 + 8
                for t in range(n_t):
                    lo = max(sq0, t * 128)
                    # ps/pt columns are sq0-relative so every matmul output
                    # stays inside one 512-col PSUM bank
                    ps = ps_s.tile([128, 1024], F32, tag="ps_s")
                    for half in range(2):
                        hl = sq0 + half * 512
                        if hl + 512 <= lo:
                            continue
                        a = max(hl, lo)
                        nc.tensor.matmul(
                            ps[:, a - sq0:hl + 512 - sq0],
                            kt2[:, t * 128:(t + 1) * 128],
                            qh[:, a:hl + 512],
                            start=True, stop=True,
                        )
                    pt = ptp.tile([128, 1024], F32R, tag="pt")
                    nc.scalar.activation(
                        pt[:, lo - sq0:1024], ps[:, lo - sq0:1024],
                        mybir.ActivationFunctionType.Exp, scale=float(SCALE),
                    )
                    if t * 128 >= sq0:
                        dc = t * 128 - sq0
                        nc.vector.tensor_mul(pt[:, dc:dc + 128], pt[:, dc:dc + 128], tm_t[:])
                    if h == 0 and c2 == 0 and t == 0:
                        nc.sync.dma_start(dbg_pt0, pt[:, :1024].bitcast(F32))
                    vt = v_all[:, t * (HD + 1):(t + 1) * (HD + 1)]
                    for half in range(2):
                        hl = sq0 + half * 512
                        if hl + 512 <= lo:
                            continue
                        a = max(hl, lo)
                        t_last = min(n_t - 1, (hl + 511) // 128)
                        nc.tensor.matmul(
                            po[:, a - sq0:hl + 512 - sq0],
                            vt, pt[:, a - sq0:hl + 512 - sq0],
                            start=(t == 0), stop=(t == t_last),
                        )
                o_sb = osb_p.tile([65, 1024], F32R, tag="osb")
                nc.vector.tensor_scalar_add(o_sb[:], po[:], 0.0)
                if h == 0 and c2 == 0:
                    nc.sync.dma_start(dbg_osb0, o_sb[:].bitcast(F32))
                d0 = rb_p.tile([1, 1024], F32R, tag="d0")
                nc.sync.dma_start(d0[:], o_sb[64:65, :])
                rb = rb_p.tile([64, 1024], F32R, tag="rb")
                for half in range(2):
                    prb = ps_sm.tile([64, 512], F32, tag="sm")
                    nc.tensor.matmul(
                        prb[:], oc_t[:], d0[:, half * 512:(half + 1) * 512],
                        start=True, stop=True,
                    )
                    nc.vector.reciprocal(rb[:, half * 512:(half + 1) * 512], prb[:])
                nc.vector.tensor_mul(
                    ot_h[h][:, sq0:sq0 + 1024], o_sb[:64, :], rb[:]
                )

        nc.sync.dma_start(dbg_qhat0, qhat[0][:].bitcast(F32))
        nc.sync.dma_start(dbg_kt2, kt2[:].bitcast(F32))
        nc.sync.dma_start(dbg_vall, v_all[:].bitcast(F32))
        nc.sync.dma_start(dbg_ot0, ot_h[0][:].bitcast(F32))

        # ---- stage D: output projection ----
        for st in range(NT):
            for n2 in range(2):
                pf = ps_sm.tile([128, 512], F32, tag="sm")
                for h in range(HPG):
                    nc.tensor.matmul(
                        pf[:], ot_h[h][:, st * 128:(st + 1) * 128],
                        wo_t[h][:, n2 * 512:(n2 + 1) * 512],
                        start=(h == 0), stop=(h == HPG - 1),
                    )
                fo = oev_p.tile([128, 512], F32, tag="fo")
                nc.vector.tensor_scalar_add(fo[:], pf[:], 0.0)
                nc.sync.dma_start(
                    out_d[st * 128:(st + 1) * 128, n2 * 512:(n2 + 1) * 512], fo[:]
                )

    nc.compile()
    _BUILD_CACHE["nc"] = nc
    return nc


def _host_inputs(x, W_qkv, W_out):
    cosT, sinT = _rope_tables()
    cos2 = np.concatenate([cosT, cosT], axis=0)          # [128, S]
    sin2 = np.concatenate([sinT, sinT], axis=0)
    rt2 = _rot_matrix()
    ident2 = np.zeros((128, HD), dtype=np.float32)
    ident2[:HD, :] = np.eye(HD, dtype=np.float32)
    ident2[HD:, :] = np.eye(HD, dtype=np.float32)
    onesc = np.ones((1, HD), dtype=np.float32)
    onesr = np.ones((128, 16), dtype=np.float32)
    # trimask[sk_local, sq_local] = 1 where sq >= sk
    tm = (np.arange(128)[None, :] >= np.arange(128)[:, None]).astype(np.float32)

    in_maps = []
    for core in range(N_CORES):
        b, g = core // NG, core % NG
        xT = np.ascontiguousarray(x[b].T)                # [D, S]
        wq = W_qkv[:, g * GW:(g + 1) * GW]               # [D, 256]
        wk = W_qkv[:, D + g * HD:D + (g + 1) * HD]       # [D, 64]
        wv = W_qkv[:, D + NG * HD + g * HD:D + NG * HD + (g + 1) * HD]
        wqkv_g = np.ascontiguousarray(
            np.concatenate([wq, wk, wv], axis=1))        # [D, 384]
        wout_g = np.ascontiguousarray(W_out[g * GW:(g + 1) * GW, :])  # [256, D]
        in_maps.append({
            "xT": xT, "wqkv": wqkv_g, "wout": wout_g,
            "cos2": cos2, "sin2": sin2, "rt2": rt2, "ident2": ident2,
            "onesc": onesc, "onesr": onesr, "trimask": tm,
        })
    return in_maps


def kernel(x, W_qkv, b_qkv, W_out, b_out, _trace=False):
    x = np.asarray(x, dtype=np.float32)
    W_qkv = np.asarray(W_qkv, dtype=np.float32)
    W_out = np.asarray(W_out, dtype=np.float32)
    b_qkv = np.asarray(b_qkv, dtype=np.float32)
    b_out = np.asarray(b_out, dtype=np.float32)
    assert not np.any(b_qkv), "nonzero b_qkv unsupported"

    nc = _build()
    in_maps = _host_inputs(x, W_qkv, W_out)
    res = run_bass_kernel_spmd(nc, in_maps, list(range(N_CORES)), trace=_trace)
    out = np.zeros((B, S, D), dtype=np.float32)
    for core in range(N_CORES):
        b = core // NG
        out[b] += res.results[core]["out"]
    out += b_out[None, None, :]
    if _trace:
        return out, res
    return out
